# revision 17
# baseline (speedup 1.0000x reference)
"""Mistral sparse-MoE (B=4,S=2048,H=1024,F=4096,E=8,top-2) on 8 trn2 cores.

Expert-parallel sharding: core e holds expert e's gate/up/down weights.
The host computes the (tiny) router + top-2 dispatch and uses it to shard:
each core receives exactly the tokens routed to its expert (gathered,
transposed, zero-padded to a common capacity C), the expert weights in
K-major partition-blocked bf16 layout, and the per-token combine weights.
The device kernel computes the full expert FFN
  y = (silu(x@gW^T) * (x@uW^T)) @ dW^T * w
for its tokens; the host scatter-adds the 8 partial outputs back into the
[T, H] result (pure unshard of the expert-parallel partial sums).

Device schedule (v2): weights are streamed exactly once (F-block outer
loop, token chunks inner), the down-projection accumulates partial sums
into an SBUF f32 accumulator, and the down-proj phase of chunk j is
emitted after the gate/up phase of chunk j+1 (software pipelining) so the
PE never head-of-line blocks on the silu/mul chain.  HBM traffic per core
is ~34 MB (vs ~101 MB for the chunk-outer schedule); the kernel is
tensor-bound at ~0.75 ms.

DRAM layouts are partition-major ([128, k, free]) so every DMA is a single
contiguous-run-per-partition access pattern.
"""

import numpy as np
import ml_dtypes
from contextlib import ExitStack

B, S, H, F, E, TOPK = 4, 2048, 1024, 4096, 8, 2
T = B * S
P = 128
NCH = 448          # token chunk (columns per psum tile, <=512)
FB = 512           # f-columns per gate/up weight block
KH = H // P        # 8  contraction chunks for gate/up
KF = F // P        # 32 contraction chunks for down
HM = H // P        # 8  output row tiles
NB = F // FB       # 8  f-blocks
KB = FB // P       # 4  fm-tiles per f-block
PD_BUFS = 3        # psum banks for the down-proj pipeline
PG_BUFS = 2        # psum banks for the gate matmul pipeline
PU_BUFS = 2        # psum banks for the up matmul pipeline
LAG = 1            # software-pipeline distance between gu and down phases
ILV = 0            # interleave gate/up k-chains (alternate psum banks per MM)

_BF16 = ml_dtypes.bfloat16


def _build_program(C, repeat=1):
    import concourse.tile as tile
    from concourse import bacc, mybir

    bf16 = mybir.dt.bfloat16
    f32 = mybir.dt.float32

    nc = bacc.Bacc("TRN2", target_bir_lowering=False, debug=False, num_devices=E)

    xT = nc.dram_tensor("xT", [P, KH, C], bf16, kind="ExternalInput").ap()
    gw = nc.dram_tensor("gw", [P, KH, F], bf16, kind="ExternalInput").ap()
    uw = nc.dram_tensor("uw", [P, KH, F], bf16, kind="ExternalInput").ap()
    dw = nc.dram_tensor("dw", [P, KF, H], bf16, kind="ExternalInput").ap()
    wr = nc.dram_tensor("wr", [P, C], f32, kind="ExternalInput").ap()
    yT = nc.dram_tensor("yT", [P, HM, C], bf16, kind="ExternalOutput").ap()

    chunks = []
    n0 = 0
    while n0 < C:
        nn = min(NCH, C - n0)
        chunks.append((n0, nn))
        n0 += nn

    with tile.TileContext(nc) as tc, ExitStack() as ctx:
        xp = ctx.enter_context(tc.tile_pool(name="xp", bufs=1))
        wp = ctx.enter_context(tc.tile_pool(name="wp", bufs=1))
        yap = ctx.enter_context(tc.tile_pool(name="yap", bufs=1))
        gwp = ctx.enter_context(tc.tile_pool(name="gwp", bufs=2))
        uwp = ctx.enter_context(tc.tile_pool(name="uwp", bufs=2))
        dwp = ctx.enter_context(tc.tile_pool(name="dwp", bufs=2))
        hp = ctx.enter_context(tc.tile_pool(name="hp", bufs=3))
        sgp = ctx.enter_context(tc.tile_pool(name="sgp", bufs=4))
        yop = ctx.enter_context(tc.tile_pool(name="yop", bufs=2))
        pg = ctx.enter_context(tc.tile_pool(name="pg", bufs=PG_BUFS, space="PSUM"))
        pu = ctx.enter_context(tc.tile_pool(name="pu", bufs=PU_BUFS, space="PSUM"))
        pd = ctx.enter_context(tc.tile_pool(name="pd", bufs=PD_BUFS, space="PSUM"))

        # token activations + combine weights resident for all passes
        xt = xp.tile([P, KH, C], bf16)
        nc.sync.dma_start(out=xt[:], in_=xT[:, :, :])
        wt = wp.tile([P, C], f32)
        nc.sync.dma_start(out=wt[:], in_=wr[:, :])
        yacc = yap.tile([P, HM, C], f32)

        PSB = 512  # psum tiles must be a full 2KB bank (512 f32)

        def gu_phase(b, n0, nn, gt, ut):
            """gate/up matmuls + silu*up -> h block for (b, chunk)."""
            hb = hp.tile([P, KB, NCH], bf16, tag="hb")
            for fm in range(KB):
                psg = pg.tile([P, PSB], f32)
                psu = pu.tile([P, PSB], f32)
                if ILV:
                    for k in range(KH):
                        nc.tensor.matmul(
                            psg[:, :nn], gt[:, k, fm * P:(fm + 1) * P],
                            xt[:, k, n0:n0 + nn],
                            start=(k == 0), stop=(k == KH - 1))
                        nc.tensor.matmul(
                            psu[:, :nn], ut[:, k, fm * P:(fm + 1) * P],
                            xt[:, k, n0:n0 + nn],
                            start=(k == 0), stop=(k == KH - 1))
                else:
                    for k in range(KH):
                        nc.tensor.matmul(
                            psg[:, :nn], gt[:, k, fm * P:(fm + 1) * P],
                            xt[:, k, n0:n0 + nn],
                            start=(k == 0), stop=(k == KH - 1))
                    for k in range(KH):
                        nc.tensor.matmul(
                            psu[:, :nn], ut[:, k, fm * P:(fm + 1) * P],
                            xt[:, k, n0:n0 + nn],
                            start=(k == 0), stop=(k == KH - 1))
                sg = sgp.tile([P, NCH], bf16)
                nc.scalar.activation(
                    sg[:, :nn], psg[:, :nn], mybir.ActivationFunctionType.Silu)
                nc.vector.tensor_mul(hb[:, fm, :nn], sg[:, :nn], psu[:, :nn])
            return hb

        def _down_evac(b, n0, nn, hm, psy):
            if b == 0:
                nc.vector.tensor_copy(yacc[:, hm, n0:n0 + nn], psy[:, :nn])
            else:
                nc.vector.tensor_add(
                    yacc[:, hm, n0:n0 + nn],
                    yacc[:, hm, n0:n0 + nn], psy[:, :nn])

        def down_phase(b, n0, nn, dt, hb):
            """partial down-proj of chunk, accumulated into yacc."""
            if ILV:
                for hm in range(0, HM, 2):
                    psy0 = pd.tile([P, PSB], f32, tag="psy", name="psy0")
                    psy1 = pd.tile([P, PSB], f32, tag="psy", name="psy1")
                    for k in range(KB):
                        nc.tensor.matmul(
                            psy0[:, :nn], dt[:, k, hm * P:(hm + 1) * P],
                            hb[:, k, :nn],
                            start=(k == 0), stop=(k == KB - 1))
                        nc.tensor.matmul(
                            psy1[:, :nn], dt[:, k, (hm + 1) * P:(hm + 2) * P],
                            hb[:, k, :nn],
                            start=(k == 0), stop=(k == KB - 1))
                    _down_evac(b, n0, nn, hm, psy0)
                    _down_evac(b, n0, nn, hm + 1, psy1)
                return
            for hm in range(HM):
                psy = pd.tile([P, PSB], f32)
                for k in range(KB):
                    nc.tensor.matmul(
                        psy[:, :nn], dt[:, k, hm * P:(hm + 1) * P],
                        hb[:, k, :nn],
                        start=(k == 0), stop=(k == KB - 1))
                _down_evac(b, n0, nn, hm, psy)

        for _ in range(repeat):
            pend = []  # (b, n0, nn, dt, hb) awaiting their down phase
            for b in range(NB):
                f0 = b * FB
                gt = gwp.tile([P, KH, FB], bf16)
                nc.sync.dma_start(out=gt[:], in_=gw[:, :, f0:f0 + FB])
                ut = uwp.tile([P, KH, FB], bf16)
                nc.sync.dma_start(out=ut[:], in_=uw[:, :, f0:f0 + FB])
                dt = dwp.tile([P, KB, H], bf16)
                nc.sync.dma_start(out=dt[:], in_=dw[:, b * KB:(b + 1) * KB, :])

                for (n0, nn) in chunks:
                    hb = gu_phase(b, n0, nn, gt, ut)
                    pend.append((b, n0, nn, dt, hb))
                    if len(pend) > LAG:
                        down_phase(*pend.pop(0))
            for args in pend:
                down_phase(*args)

            for hm in range(HM):
                yo = yop.tile([P, C], bf16, tag="yo")
                nc.vector.tensor_mul(yo[:], yacc[:, hm, :], wt[:])
                nc.sync.dma_start(out=yT[:, hm, :], in_=yo[:])

    nc.finalize()
    return nc


def _route(x, router_w):
    # top-2 routing in f64 (exactly ties-stable vs the fp32 reference for
    # any non-degenerate logits)
    logits = x.astype(np.float64) @ router_w.T.astype(np.float64)
    rows = np.arange(T)
    i1 = np.argmax(logits, axis=1)
    v1 = logits[rows, i1]
    masked = logits.copy()
    masked[rows, i1] = -np.inf
    i2 = np.argmax(masked, axis=1)
    v2 = masked[rows, i2]
    e2 = np.exp(v2 - v1)
    w1 = 1.0 / (1.0 + e2)
    w2 = e2 / (1.0 + e2)
    return i1, i2, w1.astype(np.float32), w2.astype(np.float32)


def _pmajor(a, kdim):
    """[K*128, N] -> [128, K, N] partition-major contiguous."""
    k, n = a.shape
    return np.ascontiguousarray(
        a.reshape(kdim, P, n).transpose(1, 0, 2))


SEG = 2432  # per-round token capacity bound (SBUF-resident x/yacc limit)


def kernel(hidden_states, router_w, gate_w, up_w, down_w):
    from concourse.bass_utils import run_bass_kernel_spmd

    x = np.asarray(hidden_states, dtype=np.float32).reshape(T, H)
    router_w = np.asarray(router_w, dtype=np.float32)

    i1, i2, w1, w2 = _route(x, router_w)

    idxs, wts = [], []
    for e in range(E):
        m1 = i1 == e
        m2 = i2 == e
        idx = np.nonzero(m1 | m2)[0]
        w = np.where(m1[idx], w1[idx], w2[idx])
        idxs.append(idx)
        wts.append(w)

    x_bf = x.astype(_BF16)
    w_maps = [{
        "gw": _pmajor(np.asarray(gate_w)[e].T.astype(_BF16), KH),
        "uw": _pmajor(np.asarray(up_w)[e].T.astype(_BF16), KH),
        "dw": _pmajor(np.asarray(down_w)[e].T.astype(_BF16), KF),
    } for e in range(E)]

    out = np.zeros((T, H), dtype=np.float32)
    max_ne = max(len(i) for i in idxs)
    # normally one round; multiple only under extreme routing imbalance
    # (x + f32 accumulator are SBUF-resident, capping per-run capacity)
    for r in range(0, max(1, -(-max_ne // SEG))):
        idxs_r = [idx[r * SEG:(r + 1) * SEG] for idx in idxs]
        wts_r = [w[r * SEG:(r + 1) * SEG] for w in wts]
        ne_r = max(len(i) for i in idxs_r)
        C = max(NCH, ((ne_r + P - 1) // P) * P)

        in_maps = []
        for e in range(E):
            idx, w = idxs_r[e], wts_r[e]
            n_e = len(idx)
            xTe = np.zeros((H, C), dtype=_BF16)
            xTe[:, :n_e] = x_bf[idx].T
            wre = np.zeros((P, C), dtype=np.float32)
            wre[:, :n_e] = w[None, :]
            in_maps.append({
                "xT": _pmajor(xTe, KH),
                "wr": wre,
                **w_maps[e],
            })

        nc = _build_program(C)
        results = run_bass_kernel_spmd(nc, in_maps, list(range(E))).results

        for e in range(E):
            idx = idxs_r[e]
            # yT dram is [128, HM, C] partition-major -> [H, C]
            yTe = results[e]["yT"].transpose(1, 0, 2).reshape(H, C)
            out[idx] += yTe[:, :len(idx)].astype(np.float32).T
    return out.reshape(B, S, H)
